# revision 14
# baseline (speedup 1.0000x reference)
"""Trainium2 Bass kernel for nn_NeuralDevice (segment_reduce), v5.

Architecture (per reference):
  two "eyes": h = relu(x @ Wr + br)            [N=1M, 64] -> [N, 128]
              segment-mean over idx (B=65536)  -> [B, 128]
              e = relu(mean @ Wc + bc)         -> [B, 128]
  brain:      z = [e0, e1]; out = relu(z@Wb1+bb1) @ Wb2 + bb2 -> [B, 128]

Distribution: shuffle-by-key, 8 cores x 8192 segments, with a host-side
JOINT SEGMENT PERMUTATION (pair + snake-deal balancing) that equalizes
per-(core,window) row counts across cores, minimizing SPMD padding.

Host sorts each core's nodes by (permuted) segment, prescales each row by
16/max(cnt,1) (segment SUM == 16*mean; the 16 is descaled exactly via
Wc/16) and ships x as fp8e4.  One-hot row->segment selectors are fp8
(exact 0/1).  mm1: row-tiled K=64 chunk pairs (concurrent PE tiles),
mm2: h^T @ sel accumulated per 32-seg window into a 512-seg psum group.

The PSUM->SBUF evacuations (h relu, meanT, eT, brain hT, ys) are the
machine bottleneck; they are assigned greedily between ACT and DVE using
the measured cost model ACT=(172+FD)/1.2ns, DVE=(120+FD)/0.96ns.
"""

import sys

import numpy as np
import ml_dtypes

from concourse import bass, mybir
import concourse.bacc as bacc
import concourse.tile as tile
from concourse.bass_utils import run_bass_kernel_spmd

BF16 = ml_dtypes.bfloat16
FP8 = ml_dtypes.float8_e4m3fn

B_FULL = 65536
N_FULL = 1048576
IN_NF = 64
R_OUT = 128
C_OUT = 128
BRAIN_H = 256
BRAIN_OUT = 128

CORES = 8
SEGS = B_FULL // CORES      # 8192 segments per core
WIN = 32                    # segments per accumulation window
WGRP = 512 // WIN           # windows per PSUM group (512 segs)
HB = 8                      # chunks per h-psum batch / relu batch
XCOLS = 4096                # packed-x columns per DMA tile (32 pairs)
SELCH = 128                 # jobs per sel DMA tile
MM2_SKEW = 4                # batches between mm1 and its mm2 consumption
XSCALE = 16.0               # global power-of-2 scale on x*inv (descaled in Wc)

f32 = mybir.dt.float32
bf16 = mybir.dt.bfloat16
fp8e4 = mybir.dt.float8e4
RELU = mybir.ActivationFunctionType.Relu
COPY = mybir.ActivationFunctionType.Copy


# ----------------------------------------------------------------- planning

def _make_perm(idx0, idx1):
    """Joint segment permutation: pair segments to neutralize per-eye count
    imbalance, then snake-deal pairs (desc by combined weight) into
    (window, core) bins so every window's row count is near-equal across
    cores for BOTH eyes."""
    cnt0 = np.bincount(idx0, minlength=B_FULL)
    cnt1 = np.bincount(idx1, minlength=B_FULL)
    tot = cnt0 + cnt1
    half = B_FULL // 2
    order = np.argsort(cnt0 - cnt1, kind="stable")
    a = order[:half]
    b = order[::-1][:half].copy()
    po = np.argsort(-(tot[a] + tot[b]), kind="stable")
    a = a[po]
    b = b[po]
    nbin = CORES * (SEGS // WIN)
    rounds = WIN // 2
    pair_bin = np.empty(half, np.int64)
    fwd = np.arange(nbin)
    for r in range(rounds):
        pair_bin[r * nbin:(r + 1) * nbin] = fwd if r % 2 == 0 else fwd[::-1]
    key = pair_bin * rounds + np.repeat(np.arange(rounds), nbin)
    o2 = np.argsort(key, kind="stable")
    pa = a[o2].reshape(nbin, rounds)
    pb = b[o2].reshape(nbin, rounds)
    segs_per_bin = np.concatenate([pa, pb], axis=1)          # [nbin, WIN]
    w = np.arange(nbin) // CORES
    c = np.arange(nbin) % CORES
    base = c * SEGS + w * WIN
    pos = base[:, None] + np.arange(WIN)[None, :]
    perm = np.empty(B_FULL, np.int64)
    perm[segs_per_bin.reshape(-1)] = pos.reshape(-1)
    return perm


def _plan_eye(idx):
    """Per-eye shared window schedule + per-core sorted node placement."""
    n_win = SEGS // WIN
    owner = idx // SEGS
    per_c = {}
    runs = np.zeros((CORES, n_win), np.int64)
    for c in range(CORES):
        nodes = np.flatnonzero(owner == c)
        srel = idx[nodes] - c * SEGS
        order = np.argsort(srel, kind="stable")
        nodes = nodes[order]
        srel = srel[order]
        per_c[c] = (nodes, srel)
        runs[c] = np.bincount(srel // WIN, minlength=n_win)
    win_sizes = np.maximum(runs.max(axis=0), 1)
    return win_sizes.tolist(), per_c


def _eye_sched(win_sizes):
    """Exact-size windows; chunks may straddle two windows (2 mm2 jobs).

    Returns (nchunks, jobs_of_chunk, wfirst, wlast) where
    jobs_of_chunk[c] = list of window ids with rows in chunk c, and
    wfirst/wlast map window -> first/last chunk containing its rows.
    """
    base = np.cumsum([0] + list(win_sizes))
    total = int(-(-base[-1] // 256) * 256)
    nchunks = total // 128
    jobs_of_chunk = [[] for _ in range(nchunks)]
    wfirst = {}
    wlast = {}
    for w, sz in enumerate(win_sizes):
        c0 = int(base[w]) // 128
        c1 = int(base[w] + sz - 1) // 128
        wfirst[w] = c0
        wlast[w] = c1
        for c in range(c0, c1 + 1):
            jobs_of_chunk[c].append(w)
    return nchunks, jobs_of_chunk, wfirst, wlast


# ------------------------------------------------------------ program build

_NC_CACHE = {}

# packed bf16 weight layout: [128, 1280]
_WOFF = {"wr0": 0, "wr1": 128, "wc0": 256, "wc1": 384, "wb1lo": 512,
         "wb1hi": 768, "wb2lo": 1024, "wb2hi": 1152}
_WCOLS = 1280
# packed f32 bias layout: [128, 5]
_BOFF = {"bc0": 0, "bc1": 1, "bb1a": 2, "bb1b": 3, "bb2": 4}


def _build_nc(key):
    if key in _NC_CACHE:
        return _NC_CACHE[key]
    (ws0, ws1, has_br, has_bias) = key
    win_sizes = [list(ws0), list(ws1)]
    scheds = [_eye_sched(win_sizes[0]), _eye_sched(win_sizes[1])]
    nchunks = [scheds[0][0], scheds[1][0]]
    njobs = [sum(len(js) for js in scheds[e][1]) for e in range(2)]

    nc = bacc.Bacc("TRN2", target_bir_lowering=False, debug=False)

    xp_d = [nc.dram_tensor(f"x{e}p", [128, nchunks[e] * 64], fp8e4,
                           kind="ExternalInput") for e in range(2)]
    sel_d = [nc.dram_tensor(f"sel{e}", [128, njobs[e] * WIN], fp8e4,
                            kind="ExternalInput") for e in range(2)]
    wpk_d = nc.dram_tensor("wpk", [128, _WCOLS], bf16, kind="ExternalInput")
    bpk_d = nc.dram_tensor("bpk", [128, 5], f32, kind="ExternalInput")
    if has_br:
        invr_d = [nc.dram_tensor(f"invr{e}", [1, nchunks[e] * 128], f32,
                                 kind="ExternalInput") for e in range(2)]
        br_d = [nc.dram_tensor(f"br{e}", [1, R_OUT], bf16,
                               kind="ExternalInput") for e in range(2)]
    outT_d = nc.dram_tensor("outT", [128, SEGS], bf16, kind="ExternalOutput")

    with tile.TileContext(nc) as tc:
        with tc.tile_pool(name="consts", bufs=1) as cp:
            wpk_t = cp.tile([128, _WCOLS], bf16, tag="wpk")
            bpk_t = cp.tile([128, 5], f32, tag="bpk")
            nc.sync.dma_start(out=wpk_t[:], in_=wpk_d[:])
            nc.sync.dma_start(out=bpk_t[:], in_=bpk_d[:])

            def W(name, w=128):
                o = _WOFF[name]
                return wpk_t[:, o:o + w]

            def BIAS(name):
                o = _BOFF[name]
                return bpk_t[:, o:o + 1]

            if has_br:
                br_t = [cp.tile([1, R_OUT], bf16, tag=f"br{e}",
                                name=f"br{e}t") for e in range(2)]
                for e in range(2):
                    nc.sync.dma_start(out=br_t[e][:], in_=br_d[e][:])

            eT_t = [cp.tile([128, SEGS], bf16, tag=f"eT{e}", name=f"eT{e}t")
                    for e in range(2)]

            with (
                tc.tile_pool(name="xch", bufs=4) as xpool,
                tc.tile_pool(name="selp", bufs=4) as selp,
                tc.tile_pool(name="hs", bufs=7) as hpool,
                tc.tile_pool(name="fins", bufs=2) as fs,
                tc.tile_pool(name="invp", bufs=2) as invp,
                tc.tile_pool(name="bs", bufs=3) as bs,
                tc.tile_pool(name="hps", bufs=3, space="PSUM") as hpp,
                tc.tile_pool(name="winp", bufs=1, space="PSUM") as wpp,
                tc.tile_pool(name="wcp", bufs=1, space="PSUM") as wcp,
            ):
                gi = 0                    # global batch iteration counter
                tasks = []                # (due_gi, fn) queue
                ebal = [0.0, 0.0]         # ACT / DVE modelled busy ns

                def evac(out, in_, fd, relu, bias_name=None):
                    """psum->sbuf evacuation on the less-loaded engine."""
                    ca = (172.0 + fd) / 1.2
                    cd = (120.0 + fd) / 0.96
                    use_bias = has_bias and bias_name is not None
                    if use_bias or ebal[0] + ca <= ebal[1] + cd:
                        ebal[0] += ca
                        nc.scalar.activation(
                            out=out, in_=in_, func=RELU if relu else COPY,
                            bias=BIAS(bias_name) if use_bias else 0.0)
                    else:
                        ebal[1] += cd
                        if relu:
                            nc.vector.tensor_scalar_max(out, in_, 0.0)
                        else:
                            nc.vector.tensor_copy(out, in_)

                def flush(now):
                    i = 0
                    while i < len(tasks):
                        due, fn = tasks[i]
                        if due <= now:
                            tasks.pop(i)
                            fn()
                            i = 0
                        else:
                            i += 1

                # --------------- brain (split into 3 pipeline tasks)
                def brain_a(t):
                    def fn():
                        r0 = t * 512
                        psh_a = wcp.tile([128, 512], f32, tag="pse",
                                         name=f"pha{t}")
                        nc.tensor.matmul(out=psh_a[:], lhsT=W("wb1lo"),
                                         rhs=eT_t[0][:, r0:r0 + 512],
                                         start=True, stop=False)
                        nc.tensor.matmul(out=psh_a[:], lhsT=W("wb1hi"),
                                         rhs=eT_t[1][:, r0:r0 + 512],
                                         start=False, stop=True)
                        hTa = bs.tile([128, 512], bf16, tag="hTa",
                                      name=f"hTa{t}")
                        evac(hTa[:], psh_a[:], 512, True, "bb1a")
                        fn.hTa = hTa
                    return fn

                def brain_b(t, fa):
                    def fn():
                        r0 = t * 512
                        psh_b = wcp.tile([128, 512], f32, tag="pse",
                                         name=f"phb{t}")
                        nc.tensor.matmul(
                            out=psh_b[:],
                            lhsT=wpk_t[:, _WOFF["wb1lo"] + 128:
                                       _WOFF["wb1lo"] + 256],
                            rhs=eT_t[0][:, r0:r0 + 512],
                            start=True, stop=False)
                        nc.tensor.matmul(
                            out=psh_b[:],
                            lhsT=wpk_t[:, _WOFF["wb1hi"] + 128:
                                       _WOFF["wb1hi"] + 256],
                            rhs=eT_t[1][:, r0:r0 + 512],
                            start=False, stop=True)
                        hTb = bs.tile([128, 512], bf16, tag="hTb",
                                      name=f"hTb{t}")
                        evac(hTb[:], psh_b[:], 512, True, "bb1b")
                        fn.hTb = hTb
                    return fn

                def brain_c(t, fa, fb):
                    def fn():
                        r0 = t * 512
                        psy = wcp.tile([128, 512], f32, tag="pse",
                                       name=f"py{t}")
                        nc.tensor.matmul(out=psy[:], lhsT=W("wb2lo"),
                                         rhs=fa.hTa[:], start=True,
                                         stop=False)
                        nc.tensor.matmul(out=psy[:], lhsT=W("wb2hi"),
                                         rhs=fb.hTb[:], start=False,
                                         stop=True)
                        ys = bs.tile([128, 512], bf16, tag="ys",
                                     name=f"ys{t}")
                        evac(ys[:], psy[:], 512, False, "bb2")
                        nc.sync.dma_start(out=outT_d[:, r0:r0 + 512],
                                          in_=ys[:])
                    return fn

                # --------------- eye fin: wc matmul + eT relu
                def fin_wc(e, g, meanT):
                    def fn():
                        pse = wcp.tile([128, WGRP * WIN], f32, tag="pse",
                                       name=f"pse{e}_{g}")
                        nc.tensor.matmul(out=pse[:], lhsT=W(f"wc{e}"),
                                         rhs=meanT[:], start=True, stop=True)
                        evac(eT_t[e][:, g * 512:(g + 1) * 512],
                             pse[:], 512, True, f"bc{e}")
                        if e == 1:
                            fa = brain_a(g)
                            fb = brain_b(g, fa)
                            fc = brain_c(g, fa, fb)
                            tasks.append((gi + 2, fa))
                            tasks.append((gi + 4, fb))
                            tasks.append((gi + 6, fc))
                    return fn

                for e in range(2):
                    nch, jobs_of_chunk, wfirst, wlast = scheds[e]
                    njob = njobs[e]
                    invt = None
                    wacc = None
                    jid = 0            # running job index (sel layout order)
                    pend = []          # batches awaiting mm2 emission
                    x_tiles = {}
                    sel_tiles = {}
                    npairx = (nch * 64 + XCOLS - 1) // XCOLS
                    nselt = (njob + SELCH - 1) // SELCH

                    def get_x(pi):
                        if pi not in x_tiles:
                            pbase = pi * XCOLS
                            pcsz = min(XCOLS, nch * 64 - pbase)
                            xt = xpool.tile([128, XCOLS], fp8e4, tag="xch",
                                            name=f"xch{e}_{pi}")
                            nc.sync.dma_start(
                                out=xt[:, :pcsz],
                                in_=xp_d[e][:, pbase:pbase + pcsz])
                            x_tiles[pi] = xt
                            if len(x_tiles) > 3:
                                x_tiles.pop(min(x_tiles))
                        return x_tiles[pi]

                    def get_sel(ti):
                        if ti not in sel_tiles:
                            j0 = ti * SELCH
                            scnt = min(SELCH, njob - j0)
                            st = selp.tile([128, SELCH * WIN], fp8e4,
                                           tag="selt", name=f"selt{e}_{j0}")
                            nc.sync.dma_start(
                                out=st[:, :scnt * WIN],
                                in_=sel_d[e][:, j0 * WIN:(j0 + scnt) * WIN])
                            sel_tiles[ti] = st
                            if len(sel_tiles) > 3:
                                sel_tiles.pop(min(sel_tiles))
                        return sel_tiles[ti]

                    def emit_mm2(c0, n, hsb):
                        nonlocal wacc, jid
                        for j in range(n):
                            c = c0 + j
                            slot = (j >> 1) + (j & 1) * (HB // 2)
                            for w in jobs_of_chunk[c]:
                                g = w // WGRP
                                ti = jid // SELCH
                                selt = get_sel(ti)
                                if ti + 1 < nselt:
                                    get_sel(ti + 1)
                                off = (jid % SELCH) * WIN
                                jid += 1
                                if w == g * WGRP and c == wfirst[w]:
                                    wacc = wpp.tile([128, WGRP * WIN], f32,
                                                    tag="wacc",
                                                    name=f"wa{e}_{g}")
                                ws = (w % WGRP) * WIN
                                nc.tensor.matmul(
                                    out=wacc[:, ws:ws + WIN],
                                    lhsT=hsb[:, slot * 128:(slot + 1) * 128],
                                    rhs=selt[:, off:off + WIN],
                                    start=(c == wfirst[w]),
                                    stop=(c == wlast[w]),
                                )
                                if c == wlast[w] and w % WGRP == WGRP - 1:
                                    meanT = fs.tile([128, 512], bf16,
                                                    tag="meanT",
                                                    name=f"mt{e}_{g}")
                                    evac(meanT[:], wacc[:], 512, False)
                                    tasks.append((gi + 2,
                                                  fin_wc(e, g, meanT)))

                    for c0 in range(0, nch, HB):
                        if len(pend) >= MM2_SKEW:
                            emit_mm2(*pend.pop(0))
                        flush(gi)
                        n = min(HB, nch - c0)
                        hps = hpp.tile([128, HB * 128], f32, tag="hps",
                                       name=f"hps{e}_{c0}")
                        hsb = hpool.tile([128, HB * 128], bf16, tag="hsb",
                                         name=f"hsb{e}_{c0}")
                        for t in range(n // 2):
                            pair = c0 // 2 + t
                            pi = pair // (XCOLS // 128)
                            xt = get_x(pi)
                            if pi + 1 < npairx:
                                get_x(pi + 1)
                            if has_br and pair % (XCOLS // 128) == 0:
                                ibase = pair * 256
                                icsz = min(2 * XCOLS, nch * 128 - ibase)
                                invt = invp.tile([1, 2 * XCOLS], f32,
                                                 tag="invr",
                                                 name=f"invr{e}_{pair}")
                                nc.sync.dma_start(
                                    out=invt[:, :icsz],
                                    in_=invr_d[e][:, ibase:ibase + icsz])
                            col = (pair % (XCOLS // 128)) * 128
                            for half in range(2):
                                # row-tiled pair: A -> slot t, B -> slot
                                # HB//2+t (concurrent row tiles must write
                                # different PSUM banks)
                                slot = t + half * (HB // 2)
                                hs = slice(slot * 128, (slot + 1) * 128)
                                pb = half * 64
                                nc.tensor.matmul(
                                    out=hps[:, hs],
                                    lhsT=xt[pb:pb + 64, col:col + 128],
                                    rhs=W(f"wr{e}")[pb:pb + 64, :],
                                    start=True, stop=not has_br,
                                )
                                if has_br:
                                    ic = (pair % (XCOLS // 128)) * 256 \
                                        + half * 128
                                    nc.tensor.matmul(
                                        out=hps[:, hs],
                                        lhsT=invt[0:1, ic:ic + 128],
                                        rhs=br_t[e][0:1, :],
                                        start=False, stop=True,
                                    )
                        if n == HB:
                            spans = [(0, HB * 128)]
                        else:
                            spans = [(0, (n // 2) * 128),
                                     ((HB // 2) * 128, (n // 2) * 128)]
                        for hh, hsz in spans:
                            evac(hsb[:, hh:hh + hsz], hps[:, hh:hh + hsz],
                                 hsz, True)
                        pend.append((c0, n, hsb))
                        gi += 1
                    while pend:
                        emit_mm2(*pend.pop(0))
                        flush(gi)
                        gi += 1
                while tasks:
                    gi += 1
                    flush(gi)

    nc.compile()
    _NC_CACHE[key] = nc
    return nc


# ------------------------------------------------------------------ driver

def _prepare(inputs):
    x = [np.asarray(inputs["x0"], np.float32),
         np.asarray(inputs["x1"], np.float32)]
    idx = [np.asarray(inputs["idx0"]).astype(np.int64),
           np.asarray(inputs["idx1"]).astype(np.int64)]
    br = [np.asarray(inputs["br0"], np.float32),
          np.asarray(inputs["br1"], np.float32)]
    has_br = bool(np.any(br[0]) or np.any(br[1]))
    has_bias = bool(
        np.any(np.asarray(inputs["bc0"])) or np.any(np.asarray(inputs["bc1"]))
        or np.any(np.asarray(inputs["bb1"])) or np.any(np.asarray(inputs["bb2"]))
    )

    perm = _make_perm(idx[0], idx[1])
    idxp = [perm[idx[0]], perm[idx[1]]]

    plans = [_plan_eye(idxp[0]), _plan_eye(idxp[1])]
    win_sizes = [plans[0][0], plans[1][0]]
    win_base = [np.cumsum([0] + ws) for ws in win_sizes]
    totals = [int(-(-int(win_base[e][-1]) // 256) * 256) for e in range(2)]
    n_win = SEGS // WIN
    print(f"pack: totals={totals} pad="
          f"{[t * CORES / N_FULL - 1 for t in totals]}", file=sys.stderr)
    # shared job order: (chunk asc, window asc)
    jobs = []
    for e in range(2):
        nchunks_e, jobs_of_chunk, _, _ = _eye_sched(win_sizes[e])
        jw = []
        jc = []
        for c in range(nchunks_e):
            for w in jobs_of_chunk[c]:
                jc.append(c)
                jw.append(w)
        jobs.append((np.array(jc), np.array(jw)))

    invc = [
        (XSCALE / np.maximum(
            np.bincount(idx[e], minlength=B_FULL), 1)).astype(np.float32)
        for e in range(2)
    ]

    wpk = np.zeros((128, _WCOLS), np.float32)
    for e in range(2):
        wr = np.asarray(inputs[f"Wr{e}"], np.float32)
        wpk[:, _WOFF[f"wr{e}"]:_WOFF[f"wr{e}"] + 128] = \
            np.concatenate([wr, wr], axis=0)
        wpk[:, _WOFF[f"wc{e}"]:_WOFF[f"wc{e}"] + 128] = \
            np.asarray(inputs[f"Wc{e}"], np.float32) / XSCALE
    wb1 = np.asarray(inputs["Wb1"], np.float32)
    wb2 = np.asarray(inputs["Wb2"], np.float32)
    wpk[:, _WOFF["wb1lo"]:_WOFF["wb1lo"] + 256] = wb1[0:128]
    wpk[:, _WOFF["wb1hi"]:_WOFF["wb1hi"] + 256] = wb1[128:256]
    wpk[:, _WOFF["wb2lo"]:_WOFF["wb2lo"] + 128] = wb2[0:128]
    wpk[:, _WOFF["wb2hi"]:_WOFF["wb2hi"] + 128] = wb2[128:256]

    bb1 = np.asarray(inputs["bb1"], np.float32)
    bpk = np.zeros((128, 5), np.float32)
    bpk[:, _BOFF["bc0"]] = np.asarray(inputs["bc0"], np.float32)
    bpk[:, _BOFF["bc1"]] = np.asarray(inputs["bc1"], np.float32)
    bpk[:, _BOFF["bb1a"]] = bb1[0:128]
    bpk[:, _BOFF["bb1b"]] = bb1[128:256]
    bpk[:, _BOFF["bb2"]] = np.asarray(inputs["bb2"], np.float32)

    shared = {"wpk": wpk.astype(BF16), "bpk": bpk}
    if has_br:
        for e in range(2):
            shared[f"br{e}"] = br[e].astype(BF16).reshape(1, -1)

    in_maps = []
    for c in range(CORES):
        m = dict(shared)
        for e in range(2):
            nodes, srel = plans[e][1][c]
            total = totals[e]
            nchunks = total // 128
            wid = srel // WIN
            wstart = np.searchsorted(wid, np.arange(n_win))
            pos = np.empty(len(nodes), np.int64)
            for w in range(n_win):
                lo = wstart[w]
                hi = wstart[w + 1] if w + 1 < n_win else len(nodes)
                pos[lo:hi] = win_base[e][w] + np.arange(hi - lo)
            arr = np.zeros((total, IN_NF), np.float32)
            arr[pos] = x[e][nodes] * invc[e][idx[e][nodes]][:, None]
            a3 = arr.reshape(nchunks, 128, IN_NF).astype(FP8)
            xp = np.concatenate([a3[0::2], a3[1::2]], axis=2)
            m[f"x{e}p"] = np.ascontiguousarray(
                xp.transpose(2, 0, 1).reshape(128, total // 2))
            segv = np.full(total, -10 * SEGS, np.int64)
            segv[pos] = srel
            jc, jw = jobs[e]
            # job block j: one-hot of (seg - WIN*w_j) over chunk c_j's rows
            rel = segv.reshape(nchunks, 128)[jc] - (jw * WIN)[:, None]
            sel = (rel[:, :, None] ==
                   np.arange(WIN, dtype=np.int64)[None, None, :])
            m[f"sel{e}"] = np.ascontiguousarray(
                sel.transpose(1, 0, 2).reshape(128, len(jc) * WIN)
            ).astype(FP8)
            if has_br:
                iv = np.zeros(total, np.float32)
                iv[pos] = invc[e][idx[e][nodes]]
                m[f"invr{e}"] = iv.reshape(1, total)
        in_maps.append(m)
    key = (tuple(win_sizes[0]), tuple(win_sizes[1]), has_br, has_bias)
    return key, in_maps, perm


def _axon_reset():
    try:
        import ctypes

        lib = ctypes.CDLL("/opt/axon/libaxon_pjrt.so")
        lib.axon_reset.restype = ctypes.c_int
        lib.axon_reset()
    except Exception:
        pass


def _run(inputs, trace=False, trace_kwargs=None):
    key, in_maps, perm = _prepare(inputs)
    nc = _build_nc(key)
    try:
        res = run_bass_kernel_spmd(nc, in_maps, list(range(CORES)),
                                   trace=trace, **(trace_kwargs or {}))
    except Exception as e:
        if "UNRECOVERABLE" not in str(e) and "UNAVAILABLE" not in str(e):
            raise
        _axon_reset()
        res = run_bass_kernel_spmd(nc, in_maps, list(range(CORES)),
                                   trace=trace, **(trace_kwargs or {}))
    allp = np.concatenate(
        [res.results[c]["outT"].T for c in range(CORES)], axis=0)
    out = allp[perm].astype(np.float32)
    return out, res


def kernel(**inputs):
    return _run(inputs)[0]


# revision 15
# speedup vs baseline: 1.1772x; 1.1772x over previous
"""Trainium2 Bass kernel for nn_NeuralDevice (segment_reduce), v5.

Architecture (per reference):
  two "eyes": h = relu(x @ Wr + br)            [N=1M, 64] -> [N, 128]
              segment-mean over idx (B=65536)  -> [B, 128]
              e = relu(mean @ Wc + bc)         -> [B, 128]
  brain:      z = [e0, e1]; out = relu(z@Wb1+bb1) @ Wb2 + bb2 -> [B, 128]

Distribution: shuffle-by-key, 8 cores x 8192 segments, with a host-side
JOINT SEGMENT PERMUTATION (pair + snake-deal balancing) that equalizes
per-(core,window) row counts across cores, minimizing SPMD padding.

Host sorts each core's nodes by (permuted) segment, prescales each row by
16/max(cnt,1) (segment SUM == 16*mean; the 16 is descaled exactly via
Wc/16) and ships x as fp8e4.  One-hot row->segment selectors are fp8
(exact 0/1).  mm1: row-tiled K=64 chunk pairs (concurrent PE tiles),
mm2: h^T @ sel accumulated per 32-seg window into a 512-seg psum group.

The PSUM->SBUF evacuations (h relu, meanT, eT, brain hT, ys) are the
machine bottleneck; they are assigned greedily between ACT and DVE using
the measured cost model ACT=(172+FD)/1.2ns, DVE=(120+FD)/0.96ns.
"""

import sys

import numpy as np
import ml_dtypes

from concourse import bass, mybir
import concourse.bacc as bacc
import concourse.tile as tile
from concourse.bass_utils import run_bass_kernel_spmd

BF16 = ml_dtypes.bfloat16
FP8 = ml_dtypes.float8_e4m3fn

B_FULL = 65536
N_FULL = 1048576
IN_NF = 64
R_OUT = 128
C_OUT = 128
BRAIN_H = 256
BRAIN_OUT = 128

CORES = 8
SEGS = B_FULL // CORES      # 8192 segments per core
WIN = 32                    # segments per accumulation window
WGRP = 512 // WIN           # windows per PSUM group (512 segs)
HB = 8                      # chunks per h-psum batch / relu batch
XCOLS = 4096                # packed-x columns per DMA tile (32 pairs)
SELCH = 128                 # jobs per sel DMA tile
MM2_SKEW = 4                # batches between mm1 and its mm2 consumption
XSCALE = 16.0               # global power-of-2 scale on x*inv (descaled in Wc)

f32 = mybir.dt.float32
bf16 = mybir.dt.bfloat16
fp8e4 = mybir.dt.float8e4
RELU = mybir.ActivationFunctionType.Relu
COPY = mybir.ActivationFunctionType.Copy


# ----------------------------------------------------------------- planning

def _make_perm(idx0, idx1):
    """Joint segment permutation: pair segments to neutralize per-eye count
    imbalance, then snake-deal pairs (desc by combined weight) into
    (window, core) bins so every window's row count is near-equal across
    cores for BOTH eyes."""
    cnt0 = np.bincount(idx0, minlength=B_FULL)
    cnt1 = np.bincount(idx1, minlength=B_FULL)
    tot = cnt0 + cnt1
    half = B_FULL // 2
    order = np.argsort(cnt0 - cnt1, kind="stable")
    a = order[:half]
    b = order[::-1][:half].copy()
    po = np.argsort(-(tot[a] + tot[b]), kind="stable")
    a = a[po]
    b = b[po]
    nbin = CORES * (SEGS // WIN)
    rounds = WIN // 2
    pair_bin = np.empty(half, np.int64)
    fwd = np.arange(nbin)
    for r in range(rounds):
        pair_bin[r * nbin:(r + 1) * nbin] = fwd if r % 2 == 0 else fwd[::-1]
    key = pair_bin * rounds + np.repeat(np.arange(rounds), nbin)
    o2 = np.argsort(key, kind="stable")
    pa = a[o2].reshape(nbin, rounds)
    pb = b[o2].reshape(nbin, rounds)
    segs_per_bin = np.concatenate([pa, pb], axis=1)          # [nbin, WIN]
    w = np.arange(nbin) // CORES
    c = np.arange(nbin) % CORES
    base = c * SEGS + w * WIN
    pos = base[:, None] + np.arange(WIN)[None, :]
    perm = np.empty(B_FULL, np.int64)
    perm[segs_per_bin.reshape(-1)] = pos.reshape(-1)
    return perm


def _plan_eye(idx):
    """Per-eye shared window schedule + per-core sorted node placement."""
    n_win = SEGS // WIN
    owner = idx // SEGS
    per_c = {}
    runs = np.zeros((CORES, n_win), np.int64)
    for c in range(CORES):
        nodes = np.flatnonzero(owner == c)
        srel = idx[nodes] - c * SEGS
        order = np.argsort(srel, kind="stable")
        nodes = nodes[order]
        srel = srel[order]
        per_c[c] = (nodes, srel)
        runs[c] = np.bincount(srel // WIN, minlength=n_win)
    win_sizes = np.maximum(runs.max(axis=0), 1)
    return win_sizes.tolist(), per_c


def _eye_sched(win_sizes):
    """Exact-size windows; chunks may straddle two windows (2 mm2 jobs).

    Returns (nchunks, jobs_of_chunk, wfirst, wlast) where
    jobs_of_chunk[c] = list of window ids with rows in chunk c, and
    wfirst/wlast map window -> first/last chunk containing its rows.
    """
    base = np.cumsum([0] + list(win_sizes))
    total = int(-(-base[-1] // 256) * 256)
    nchunks = total // 128
    jobs_of_chunk = [[] for _ in range(nchunks)]
    wfirst = {}
    wlast = {}
    for w, sz in enumerate(win_sizes):
        c0 = int(base[w]) // 128
        c1 = int(base[w] + sz - 1) // 128
        wfirst[w] = c0
        wlast[w] = c1
        for c in range(c0, c1 + 1):
            jobs_of_chunk[c].append(w)
    return nchunks, jobs_of_chunk, wfirst, wlast


# ------------------------------------------------------------ program build

_NC_CACHE = {}

# packed bf16 weight layout: [128, 1280]
_WOFF = {"wr0": 0, "wr1": 128, "wc0": 256, "wc1": 384, "wb1lo": 512,
         "wb1hi": 768, "wb2lo": 1024, "wb2hi": 1152}
_WCOLS = 1280
# packed f32 bias layout: [128, 5]
_BOFF = {"bc0": 0, "bc1": 1, "bb1a": 2, "bb1b": 3, "bb2": 4}


def _build_nc(key):
    if key in _NC_CACHE:
        return _NC_CACHE[key]
    (ws0, ws1, has_br, has_bias) = key
    win_sizes = [list(ws0), list(ws1)]
    scheds = [_eye_sched(win_sizes[0]), _eye_sched(win_sizes[1])]
    nchunks = [scheds[0][0], scheds[1][0]]
    njobs = [sum(len(js) for js in scheds[e][1]) for e in range(2)]

    nc = bacc.Bacc("TRN2", target_bir_lowering=False, debug=False)

    xp_d = [nc.dram_tensor(f"x{e}p", [128, nchunks[e] * 64], fp8e4,
                           kind="ExternalInput") for e in range(2)]
    sel_d = [nc.dram_tensor(f"sel{e}", [128, njobs[e] * WIN], fp8e4,
                            kind="ExternalInput") for e in range(2)]
    wpk_d = nc.dram_tensor("wpk", [128, _WCOLS], bf16, kind="ExternalInput")
    bpk_d = nc.dram_tensor("bpk", [128, 5], f32, kind="ExternalInput")
    if has_br:
        invr_d = [nc.dram_tensor(f"invr{e}", [1, nchunks[e] * 128], f32,
                                 kind="ExternalInput") for e in range(2)]
        br_d = [nc.dram_tensor(f"br{e}", [1, R_OUT], bf16,
                               kind="ExternalInput") for e in range(2)]
    outT_d = nc.dram_tensor("outT", [128, SEGS], bf16, kind="ExternalOutput")

    with tile.TileContext(nc) as tc:
        with tc.tile_pool(name="consts", bufs=1) as cp:
            wpk_t = cp.tile([128, _WCOLS], bf16, tag="wpk")
            bpk_t = cp.tile([128, 5], f32, tag="bpk")
            nc.sync.dma_start(out=wpk_t[:], in_=wpk_d[:])
            nc.sync.dma_start(out=bpk_t[:], in_=bpk_d[:])

            def W(name, w=128):
                o = _WOFF[name]
                return wpk_t[:, o:o + w]

            def BIAS(name):
                o = _BOFF[name]
                return bpk_t[:, o:o + 1]

            if has_br:
                br_t = [cp.tile([1, R_OUT], bf16, tag=f"br{e}",
                                name=f"br{e}t") for e in range(2)]
                for e in range(2):
                    nc.sync.dma_start(out=br_t[e][:], in_=br_d[e][:])

            eT_t = [cp.tile([128, SEGS], bf16, tag=f"eT{e}", name=f"eT{e}t")
                    for e in range(2)]

            with (
                tc.tile_pool(name="xch", bufs=4) as xpool,
                tc.tile_pool(name="selp", bufs=4) as selp,
                tc.tile_pool(name="hs", bufs=7) as hpool,
                tc.tile_pool(name="fins", bufs=2) as fs,
                tc.tile_pool(name="invp", bufs=2) as invp,
                tc.tile_pool(name="bs", bufs=3) as bs,
                tc.tile_pool(name="hps", bufs=3, space="PSUM") as hpp,
                tc.tile_pool(name="winp", bufs=1, space="PSUM") as wpp,
                tc.tile_pool(name="wcp", bufs=1, space="PSUM") as wcp,
            ):
                gi = 0                    # global batch iteration counter
                tasks = []                # (due_gi, fn) queue
                ebal = [0.0, 0.0]         # ACT / DVE modelled busy ns

                def evac(out, in_, fd, relu, bias_name=None):
                    """psum->sbuf evacuation on the less-loaded engine."""
                    ca = (172.0 + fd) / 1.2
                    cd = (120.0 + fd) / 0.96
                    use_bias = has_bias and bias_name is not None
                    if use_bias or ebal[0] + ca <= ebal[1] + cd:
                        ebal[0] += ca
                        nc.scalar.activation(
                            out=out, in_=in_, func=RELU if relu else COPY,
                            bias=BIAS(bias_name) if use_bias else 0.0)
                    else:
                        ebal[1] += cd
                        if relu:
                            nc.vector.tensor_scalar_max(out, in_, 0.0)
                        else:
                            nc.vector.tensor_copy(out, in_)

                def flush(now):
                    i = 0
                    while i < len(tasks):
                        due, fn = tasks[i]
                        if due <= now:
                            tasks.pop(i)
                            fn()
                            i = 0
                        else:
                            i += 1

                # --------------- brain (split into 3 pipeline tasks)
                def brain_a(t):
                    def fn():
                        r0 = t * 512
                        psh_a = wcp.tile([128, 512], f32, tag="pse",
                                         name=f"pha{t}")
                        nc.tensor.matmul(out=psh_a[:], lhsT=W("wb1lo"),
                                         rhs=eT_t[0][:, r0:r0 + 512],
                                         start=True, stop=False)
                        nc.tensor.matmul(out=psh_a[:], lhsT=W("wb1hi"),
                                         rhs=eT_t[1][:, r0:r0 + 512],
                                         start=False, stop=True)
                        hTa = bs.tile([128, 512], bf16, tag="hTa",
                                      name=f"hTa{t}")
                        evac(hTa[:], psh_a[:], 512, True, "bb1a")
                        fn.hTa = hTa
                    return fn

                def brain_b(t, fa):
                    def fn():
                        r0 = t * 512
                        psh_b = wcp.tile([128, 512], f32, tag="pse",
                                         name=f"phb{t}")
                        nc.tensor.matmul(
                            out=psh_b[:],
                            lhsT=wpk_t[:, _WOFF["wb1lo"] + 128:
                                       _WOFF["wb1lo"] + 256],
                            rhs=eT_t[0][:, r0:r0 + 512],
                            start=True, stop=False)
                        nc.tensor.matmul(
                            out=psh_b[:],
                            lhsT=wpk_t[:, _WOFF["wb1hi"] + 128:
                                       _WOFF["wb1hi"] + 256],
                            rhs=eT_t[1][:, r0:r0 + 512],
                            start=False, stop=True)
                        hTb = bs.tile([128, 512], bf16, tag="hTb",
                                      name=f"hTb{t}")
                        evac(hTb[:], psh_b[:], 512, True, "bb1b")
                        fn.hTb = hTb
                    return fn

                def brain_c(t, fa, fb):
                    def fn():
                        r0 = t * 512
                        psy = wcp.tile([128, 512], f32, tag="pse",
                                       name=f"py{t}")
                        nc.tensor.matmul(out=psy[:], lhsT=W("wb2lo"),
                                         rhs=fa.hTa[:], start=True,
                                         stop=False)
                        nc.tensor.matmul(out=psy[:], lhsT=W("wb2hi"),
                                         rhs=fb.hTb[:], start=False,
                                         stop=True)
                        ys = bs.tile([128, 512], bf16, tag="ys",
                                     name=f"ys{t}")
                        evac(ys[:], psy[:], 512, False, "bb2")
                        nc.sync.dma_start(out=outT_d[:, r0:r0 + 512],
                                          in_=ys[:])
                    return fn

                # --------------- eye fin: wc matmul + eT relu
                def fin_wc(e, g, meanT):
                    def fn():
                        pse = wcp.tile([128, WGRP * WIN], f32, tag="pse",
                                       name=f"pse{e}_{g}")
                        nc.tensor.matmul(out=pse[:], lhsT=W(f"wc{e}"),
                                         rhs=meanT[:], start=True, stop=True)
                        evac(eT_t[e][:, g * 512:(g + 1) * 512],
                             pse[:], 512, True, f"bc{e}")
                        if e == 1:
                            fa = brain_a(g)
                            fb = brain_b(g, fa)
                            fc = brain_c(g, fa, fb)
                            tasks.append((gi + 2, fa))
                            tasks.append((gi + 4, fb))
                            tasks.append((gi + 6, fc))
                    return fn

                for e in range(2):
                    nch, jobs_of_chunk, wfirst, wlast = scheds[e]
                    njob = njobs[e]
                    invt = None
                    wacc = None
                    jid = 0            # running job index (sel layout order)
                    pend = []          # batches awaiting mm2 emission
                    x_tiles = {}
                    sel_tiles = {}
                    npairx = (nch * 64 + XCOLS - 1) // XCOLS
                    nselt = (njob + SELCH - 1) // SELCH

                    def get_x(pi):
                        if pi not in x_tiles:
                            pbase = pi * XCOLS
                            pcsz = min(XCOLS, nch * 64 - pbase)
                            xt = xpool.tile([128, XCOLS], fp8e4, tag="xch",
                                            name=f"xch{e}_{pi}")
                            nc.sync.dma_start(
                                out=xt[:, :pcsz],
                                in_=xp_d[e][:, pbase:pbase + pcsz])
                            x_tiles[pi] = xt
                            if len(x_tiles) > 3:
                                x_tiles.pop(min(x_tiles))
                        return x_tiles[pi]

                    def get_sel(ti):
                        if ti not in sel_tiles:
                            j0 = ti * SELCH
                            scnt = min(SELCH, njob - j0)
                            st = selp.tile([128, SELCH * WIN], fp8e4,
                                           tag="selt", name=f"selt{e}_{j0}")
                            nc.sync.dma_start(
                                out=st[:, :scnt * WIN],
                                in_=sel_d[e][:, j0 * WIN:(j0 + scnt) * WIN])
                            sel_tiles[ti] = st
                            if len(sel_tiles) > 3:
                                sel_tiles.pop(min(sel_tiles))
                        return sel_tiles[ti]

                    def emit_mm2(c0, n, hsb):
                        nonlocal wacc, jid
                        for j in range(n):
                            c = c0 + j
                            slot = (j >> 1) + (j & 1) * (HB // 2)
                            for w in jobs_of_chunk[c]:
                                g = w // WGRP
                                ti = jid // SELCH
                                selt = get_sel(ti)
                                if ti + 1 < nselt:
                                    get_sel(ti + 1)
                                off = (jid % SELCH) * WIN
                                jid += 1
                                if w == g * WGRP and c == wfirst[w]:
                                    wacc = wpp.tile([128, WGRP * WIN], f32,
                                                    tag="wacc",
                                                    name=f"wa{e}_{g}")
                                ws = (w % WGRP) * WIN
                                nc.tensor.matmul(
                                    out=wacc[:, ws:ws + WIN],
                                    lhsT=hsb[:, slot * 128:(slot + 1) * 128],
                                    rhs=selt[:, off:off + WIN],
                                    start=(c == wfirst[w]),
                                    stop=(c == wlast[w]),
                                )
                                if c == wlast[w] and w % WGRP == WGRP - 1:
                                    meanT = fs.tile([128, 512], bf16,
                                                    tag="meanT",
                                                    name=f"mt{e}_{g}")
                                    evac(meanT[:], wacc[:], 512, False)
                                    tasks.append((gi + 2,
                                                  fin_wc(e, g, meanT)))

                    for c0 in range(0, nch, HB):
                        n = min(HB, nch - c0)
                        hps = hpp.tile([128, HB * 128], f32, tag="hps",
                                       name=f"hps{e}_{c0}")
                        hsb = hpool.tile([128, HB * 128], bf16, tag="hsb",
                                         name=f"hsb{e}_{c0}")
                        for t in range(n // 2):
                            pair = c0 // 2 + t
                            pi = pair // (XCOLS // 128)
                            xt = get_x(pi)
                            if pi + 1 < npairx:
                                get_x(pi + 1)
                            if has_br and pair % (XCOLS // 128) == 0:
                                ibase = pair * 256
                                icsz = min(2 * XCOLS, nch * 128 - ibase)
                                invt = invp.tile([1, 2 * XCOLS], f32,
                                                 tag="invr",
                                                 name=f"invr{e}_{pair}")
                                nc.sync.dma_start(
                                    out=invt[:, :icsz],
                                    in_=invr_d[e][:, ibase:ibase + icsz])
                            col = (pair % (XCOLS // 128)) * 128
                            for half in range(2):
                                # row-tiled pair: A -> slot t, B -> slot
                                # HB//2+t (concurrent row tiles must write
                                # different PSUM banks)
                                slot = t + half * (HB // 2)
                                hs = slice(slot * 128, (slot + 1) * 128)
                                pb = half * 64
                                nc.tensor.matmul(
                                    out=hps[:, hs],
                                    lhsT=xt[pb:pb + 64, col:col + 128],
                                    rhs=W(f"wr{e}")[pb:pb + 64, :],
                                    start=True, stop=not has_br,
                                )
                                if has_br:
                                    ic = (pair % (XCOLS // 128)) * 256 \
                                        + half * 128
                                    nc.tensor.matmul(
                                        out=hps[:, hs],
                                        lhsT=invt[0:1, ic:ic + 128],
                                        rhs=br_t[e][0:1, :],
                                        start=False, stop=True,
                                    )
                        if n == HB:
                            spans = [(0, HB * 128)]
                        else:
                            spans = [(0, (n // 2) * 128),
                                     ((HB // 2) * 128, (n // 2) * 128)]
                        for hh, hsz in spans:
                            evac(hsb[:, hh:hh + hsz], hps[:, hh:hh + hsz],
                                 hsz, True)
                        pend.append((c0, n, hsb))
                        flush(gi)
                        if len(pend) > MM2_SKEW:
                            emit_mm2(*pend.pop(0))
                        gi += 1
                    while pend:
                        emit_mm2(*pend.pop(0))
                        flush(gi)
                        gi += 1
                while tasks:
                    gi += 1
                    flush(gi)

    nc.compile()
    _NC_CACHE[key] = nc
    return nc


# ------------------------------------------------------------------ driver

def _prepare(inputs):
    x = [np.asarray(inputs["x0"], np.float32),
         np.asarray(inputs["x1"], np.float32)]
    idx = [np.asarray(inputs["idx0"]).astype(np.int64),
           np.asarray(inputs["idx1"]).astype(np.int64)]
    br = [np.asarray(inputs["br0"], np.float32),
          np.asarray(inputs["br1"], np.float32)]
    has_br = bool(np.any(br[0]) or np.any(br[1]))
    has_bias = bool(
        np.any(np.asarray(inputs["bc0"])) or np.any(np.asarray(inputs["bc1"]))
        or np.any(np.asarray(inputs["bb1"])) or np.any(np.asarray(inputs["bb2"]))
    )

    perm = _make_perm(idx[0], idx[1])
    idxp = [perm[idx[0]], perm[idx[1]]]

    plans = [_plan_eye(idxp[0]), _plan_eye(idxp[1])]
    win_sizes = [plans[0][0], plans[1][0]]
    win_base = [np.cumsum([0] + ws) for ws in win_sizes]
    totals = [int(-(-int(win_base[e][-1]) // 256) * 256) for e in range(2)]
    n_win = SEGS // WIN
    print(f"pack: totals={totals} pad="
          f"{[t * CORES / N_FULL - 1 for t in totals]}", file=sys.stderr)
    # shared job order: (chunk asc, window asc)
    jobs = []
    for e in range(2):
        nchunks_e, jobs_of_chunk, _, _ = _eye_sched(win_sizes[e])
        jw = []
        jc = []
        for c in range(nchunks_e):
            for w in jobs_of_chunk[c]:
                jc.append(c)
                jw.append(w)
        jobs.append((np.array(jc), np.array(jw)))

    invc = [
        (XSCALE / np.maximum(
            np.bincount(idx[e], minlength=B_FULL), 1)).astype(np.float32)
        for e in range(2)
    ]

    wpk = np.zeros((128, _WCOLS), np.float32)
    for e in range(2):
        wr = np.asarray(inputs[f"Wr{e}"], np.float32)
        wpk[:, _WOFF[f"wr{e}"]:_WOFF[f"wr{e}"] + 128] = \
            np.concatenate([wr, wr], axis=0)
        wpk[:, _WOFF[f"wc{e}"]:_WOFF[f"wc{e}"] + 128] = \
            np.asarray(inputs[f"Wc{e}"], np.float32) / XSCALE
    wb1 = np.asarray(inputs["Wb1"], np.float32)
    wb2 = np.asarray(inputs["Wb2"], np.float32)
    wpk[:, _WOFF["wb1lo"]:_WOFF["wb1lo"] + 256] = wb1[0:128]
    wpk[:, _WOFF["wb1hi"]:_WOFF["wb1hi"] + 256] = wb1[128:256]
    wpk[:, _WOFF["wb2lo"]:_WOFF["wb2lo"] + 128] = wb2[0:128]
    wpk[:, _WOFF["wb2hi"]:_WOFF["wb2hi"] + 128] = wb2[128:256]

    bb1 = np.asarray(inputs["bb1"], np.float32)
    bpk = np.zeros((128, 5), np.float32)
    bpk[:, _BOFF["bc0"]] = np.asarray(inputs["bc0"], np.float32)
    bpk[:, _BOFF["bc1"]] = np.asarray(inputs["bc1"], np.float32)
    bpk[:, _BOFF["bb1a"]] = bb1[0:128]
    bpk[:, _BOFF["bb1b"]] = bb1[128:256]
    bpk[:, _BOFF["bb2"]] = np.asarray(inputs["bb2"], np.float32)

    shared = {"wpk": wpk.astype(BF16), "bpk": bpk}
    if has_br:
        for e in range(2):
            shared[f"br{e}"] = br[e].astype(BF16).reshape(1, -1)

    in_maps = []
    for c in range(CORES):
        m = dict(shared)
        for e in range(2):
            nodes, srel = plans[e][1][c]
            total = totals[e]
            nchunks = total // 128
            wid = srel // WIN
            wstart = np.searchsorted(wid, np.arange(n_win))
            pos = np.empty(len(nodes), np.int64)
            for w in range(n_win):
                lo = wstart[w]
                hi = wstart[w + 1] if w + 1 < n_win else len(nodes)
                pos[lo:hi] = win_base[e][w] + np.arange(hi - lo)
            arr = np.zeros((total, IN_NF), np.float32)
            arr[pos] = x[e][nodes] * invc[e][idx[e][nodes]][:, None]
            a3 = arr.reshape(nchunks, 128, IN_NF).astype(FP8)
            xp = np.concatenate([a3[0::2], a3[1::2]], axis=2)
            m[f"x{e}p"] = np.ascontiguousarray(
                xp.transpose(2, 0, 1).reshape(128, total // 2))
            segv = np.full(total, -10 * SEGS, np.int64)
            segv[pos] = srel
            jc, jw = jobs[e]
            # job block j: one-hot of (seg - WIN*w_j) over chunk c_j's rows
            rel = segv.reshape(nchunks, 128)[jc] - (jw * WIN)[:, None]
            sel = (rel[:, :, None] ==
                   np.arange(WIN, dtype=np.int64)[None, None, :])
            m[f"sel{e}"] = np.ascontiguousarray(
                sel.transpose(1, 0, 2).reshape(128, len(jc) * WIN)
            ).astype(FP8)
            if has_br:
                iv = np.zeros(total, np.float32)
                iv[pos] = invc[e][idx[e][nodes]]
                m[f"invr{e}"] = iv.reshape(1, total)
        in_maps.append(m)
    key = (tuple(win_sizes[0]), tuple(win_sizes[1]), has_br, has_bias)
    return key, in_maps, perm


def _axon_reset():
    try:
        import ctypes

        lib = ctypes.CDLL("/opt/axon/libaxon_pjrt.so")
        lib.axon_reset.restype = ctypes.c_int
        lib.axon_reset()
    except Exception:
        pass


def _run(inputs, trace=False, trace_kwargs=None):
    key, in_maps, perm = _prepare(inputs)
    nc = _build_nc(key)
    try:
        res = run_bass_kernel_spmd(nc, in_maps, list(range(CORES)),
                                   trace=trace, **(trace_kwargs or {}))
    except Exception as e:
        if "UNRECOVERABLE" not in str(e) and "UNAVAILABLE" not in str(e):
            raise
        _axon_reset()
        res = run_bass_kernel_spmd(nc, in_maps, list(range(CORES)),
                                   trace=trace, **(trace_kwargs or {}))
    allp = np.concatenate(
        [res.results[c]["outT"].T for c in range(CORES)], axis=0)
    out = allp[perm].astype(np.float32)
    return out, res


def kernel(**inputs):
    return _run(inputs)[0]
